# revision 97
# baseline (speedup 1.0000x reference)
"""Bidirectional cross-attention Trainium2 kernel (8 NeuronCores, SPMD).

Sharding: 2 heads per core (16 heads / 8 cores); both batches on every core.
Host pre-transposes x/context to [D, B*N] bf16, slices per-head weight columns.

Device pipeline (v10):
  P1   qk/cqk projected straight into [hd, pos] (weight-stationary); v/cv
       projected straight into natural [pos, vd] layout (xT-chunk-stationary,
       weight-moving) -- no staging transposes at all.
  ATT  one pass per (batch, head), software-pipelined (sim leads 1 jw-unit;
       pv lags 4 in unit 0 to cover DMA-eT latency, 2 elsewhere): sim -> 2x [128,512] PSUM (3-rotation), one exp per
       half on ACT; e transposed on PE through two alternating PSUM banks
       (unit 0, collective-free, uses the DMA xbar instead -- tile serializes
       DmaTransposeAnt against every in-flight AllToAll, so xbar eT is only
       viable before the first A2A); ctx-dir accumulates 16x 64-wide chunks
       in two banks plus 1-col colsum matmuls; out-dir pv uses eT-stationary
       65-wide moving (values + ones column).
  A2A  one AllToAll per (batch, head) [2048 pos, o64|c64] bf16 (fp8 payload
       fails the 2e-2 gate). The post-A2A cc load + xbar block-transpose ride
       the SP queue from fire time so they run back-to-back at landing;
       merged-head copies go to the idle gpsimd engine mid-stream.
  TAIL batch-0 output projections run under the last A2A's flight; the
       (1,1) posts chain is split by token-half so each half of batch-1's
       projections starts as soon as its half lands.
"""

import numpy as np
import ml_dtypes

BF16 = ml_dtypes.bfloat16

# problem constants (hardcoded per contract)
B = 2
N = 2048
D = 1024
HEADS = 16
DIM_HEAD = 64
SCALE = DIM_HEAD ** -0.5

N_CORES = 8
HD = 128            # per-core head dims (2 heads x 64)
A = B * N           # 4096 flattened tokens
SL = N // N_CORES   # 256 per-batch output slice per core
KC = D // 128       # 8 contraction chunks for projections
AC_W = 512          # projection output chunk width
NB = A // 128       # 32 position blocks of 128
NBB = N // 128      # 16 position blocks per batch


def build_nc(reps=1, use_cc=True, debug=False, fp8_a2a=False):
    import concourse.bacc as bacc
    import concourse.tile as tile
    from concourse import mybir
    from concourse.masks import make_identity

    fp32 = mybir.dt.float32
    bf16 = mybir.dt.bfloat16
    fp8 = mybir.dt.float8e4
    EXP = mybir.ActivationFunctionType.Exp
    A2A_DT = fp8 if fp8_a2a else bf16

    nc = bacc.Bacc("TRN2", target_bir_lowering=False, num_devices=N_CORES)

    # ---- I/O ----
    xT = nc.dram_tensor("xT", [D, A], bf16, kind="ExternalInput")
    cT = nc.dram_tensor("cT", [D, A], bf16, kind="ExternalInput")
    wqk = nc.dram_tensor("wqk", [D, HD], bf16, kind="ExternalInput")
    wv = nc.dram_tensor("wv", [D, HD], bf16, kind="ExternalInput")
    wcqk = nc.dram_tensor("wcqk", [D, HD], bf16, kind="ExternalInput")
    wcv = nc.dram_tensor("wcv", [D, HD], bf16, kind="ExternalInput")
    wout = nc.dram_tensor("wout", [D, D], bf16, kind="ExternalInput")
    wcout = nc.dram_tensor("wcout", [D, D], bf16, kind="ExternalInput")
    out_sl = nc.dram_tensor("out_sl", [2 * SL, D], bf16, kind="ExternalOutput")
    ctx_sl = nc.dram_tensor("ctx_sl", [2 * SL, D], bf16, kind="ExternalOutput")

    # collective buffers, one per (batch, head): rows = pos within batch,
    # cols = [out-dir 64 | ctx-dir 64]
    cc_in = [[nc.dram_tensor(f"cc_in_{b}{h}", [N, HD], A2A_DT) for h in range(2)]
             for b in range(B)]
    cc_out = [[nc.dram_tensor(f"cc_out_{b}{h}", [N, HD], A2A_DT) for h in range(2)]
              for b in range(B)]

    with tile.TileContext(nc) as tc:
        with tc.tile_pool(name="singles", bufs=1) as singles:
            # ---- long-lived SBUF tensors ----
            wqk_sb = singles.tile([128, KC, HD], bf16)
            wv_sb = singles.tile([128, KC, HD], bf16)
            wcqk_sb = singles.tile([128, KC, HD], bf16)
            wcv_sb = singles.tile([128, KC, HD], bf16)

            qkT_sb = singles.tile([128, A], bf16)     # [hd, pos]
            cqkT_sb = singles.tile([128, A], bf16)
            # natural-layout values with fused ones column
            v0_sb = singles.tile([128, NB, 65], bf16)
            v1_sb = singles.tile([128, NB, 65], bf16)
            cv0_sb = singles.tile([128, NB, 65], bf16)
            cv1_sb = singles.tile([128, NB, 65], bf16)
            for t in (v0_sb, v1_sb, cv0_sb, cv1_sb):
                nc.vector.memset(t, 1.0)

            wout_sb = singles.tile([128, KC, D], bf16)
            wcout_sb = singles.tile([128, KC, D], bf16)

            ident = singles.tile([128, 128], bf16)
            make_identity(nc, ident)

            # natural-layout attention outputs [pos, blk, d] per head
            oa_nat = [singles.tile([128, NB, 64], A2A_DT, name=f"oa_nat{i}") for i in range(2)]
            ca_nat = [singles.tile([128, NB, 64], A2A_DT, name=f"ca_nat{i}") for i in range(2)]

            # post-A2A merged-head tiles [d%128, kc, tok] per (batch, dir)
            full_o = [singles.tile([128, KC, 2 * 128], bf16, name=f"full_o{i}") for i in range(B)]
            full_c = [singles.tile([128, KC, 2 * 128], bf16, name=f"full_c{i}") for i in range(B)]

            for _rep in range(reps):
                # ================= P1: batch-0 projections =================
                # qk/cqk produced transposed [hd, pos]; v/cv produced directly
                # in natural [pos, vd] layout via xT-chunk-stationary matmuls
                # (no vT staging / xbar transposes needed). Batch-1 windows
                # are deferred into the A2A(0,0) flight window inside ATT,
                # where the PE would otherwise stall (DmaTransposeAnt is
                # serialized against in-flight collectives).
                xv = xT.ap().rearrange("(k p) a -> p k a", p=128)
                cv_ = cT.ap().rearrange("(k p) a -> p k a", p=128)

                def p1_window(ac, x_t, c_t, qk_ps, cv2_ps, v_ps, cv_ps):
                    # x-only work first (qk, v), c-only after: the next
                    # window's x tile alone unblocks its first half.
                    for kc in range(KC):
                        st = kc == 0
                        sp = kc == KC - 1
                        nc.tensor.matmul(qk_ps, wqk_sb[:, kc, :], x_t[:, kc, :], start=st, stop=sp)
                        # v natural: stationary = xT chunk, moving = weight
                        for blk in range(4):
                            bs = slice(blk * 128, (blk + 1) * 128)
                            nc.tensor.matmul(
                                v_ps[:, bs], x_t[:, kc, bs], wv_sb[:, kc, :],
                                start=(st and blk == 0), stop=(sp and blk == 3),
                                skip_group_check=True)
                    for kc in range(KC):
                        st = kc == 0
                        sp = kc == KC - 1
                        nc.tensor.matmul(cv2_ps, wcqk_sb[:, kc, :], c_t[:, kc, :], start=st, stop=sp)
                        for blk in range(4):
                            bs = slice(blk * 128, (blk + 1) * 128)
                            nc.tensor.matmul(
                                cv_ps[:, bs], c_t[:, kc, bs], wcv_sb[:, kc, :],
                                start=(st and blk == 0), stop=(sp and blk == 3),
                                skip_group_check=True)
                    sl_ = slice(ac * AC_W, (ac + 1) * AC_W)
                    nc.vector.tensor_copy(qkT_sb[:, sl_], qk_ps)
                    nc.scalar.copy(cqkT_sb[:, sl_], cv2_ps)
                    # strided head-split copies into the 65-wide value tiles
                    bv = slice(ac * 4, ac * 4 + 4)
                    vp4 = v_ps.rearrange("p (b m) -> p b m", m=128)
                    cp4 = cv_ps.rearrange("p (b m) -> p b m", m=128)
                    nc.vector.tensor_copy(v0_sb[:, bv, 0:64], vp4[:, :, 0:64])
                    nc.vector.tensor_copy(v1_sb[:, bv, 0:64], vp4[:, :, 64:128])
                    nc.vector.tensor_copy(cv0_sb[:, bv, 0:64], cp4[:, :, 0:64])
                    nc.vector.tensor_copy(cv1_sb[:, bv, 0:64], cp4[:, :, 64:128])

                with (
                    tc.tile_pool(name="p1in", bufs=3) as p1in,
                    tc.tile_pool(name="ps1", bufs=8, space="PSUM") as ps1,
                ):
                    n_ac = A // AC_W
                    # first-needed-first DMA order: x/c kc0 of window 0, then
                    # weight kc0 chunks, then the rest.
                    x_ts, c_ts = [], []
                    for ac in range(n_ac):
                        x_ts.append(p1in.tile([128, KC, AC_W], bf16, tag="xin", name=f"x_t{ac}"))
                        c_ts.append(p1in.tile([128, KC, AC_W], bf16, tag="cin", name=f"c_t{ac}"))
                    sl0 = slice(0, AC_W)
                    nc.sync.dma_start(out=x_ts[0][:, 0:1, :], in_=xv[:, 0:1, sl0])
                    nc.sync.dma_start(out=c_ts[0][:, 0:1, :], in_=cv_[:, 0:1, sl0])
                    for w_dram, w_sb in ((wqk, wqk_sb), (wv, wv_sb),
                                         (wcqk, wcqk_sb), (wcv, wcv_sb)):
                        wvw = w_dram.ap().rearrange("(k p) m -> p k m", p=128)
                        nc.scalar.dma_start(out=w_sb[:, 0:1, :], in_=wvw[:, 0:1, :])
                    nc.sync.dma_start(out=x_ts[0][:, 1:KC, :], in_=xv[:, 1:KC, sl0])
                    for w_dram, w_sb in ((wqk, wqk_sb), (wv, wv_sb)):
                        wvw = w_dram.ap().rearrange("(k p) m -> p k m", p=128)
                        nc.scalar.dma_start(out=w_sb[:, 1:KC, :], in_=wvw[:, 1:KC, :])
                    nc.sync.dma_start(out=c_ts[0][:, 1:KC, :], in_=cv_[:, 1:KC, sl0])
                    for w_dram, w_sb in ((wcqk, wcqk_sb), (wcv, wcv_sb)):
                        wvw = w_dram.ap().rearrange("(k p) m -> p k m", p=128)
                        nc.scalar.dma_start(out=w_sb[:, 1:KC, :], in_=wvw[:, 1:KC, :])

                    for ac in range(n_ac):
                        if ac > 0:
                            slc = slice(ac * AC_W, (ac + 1) * AC_W)
                            nc.sync.dma_start(out=x_ts[ac], in_=xv[:, :, slc])
                            nc.sync.dma_start(out=c_ts[ac], in_=cv_[:, :, slc])
                        qk_ps = ps1.tile([128, AC_W], fp32, tag="p1")
                        cv2_ps = ps1.tile([128, AC_W], fp32, tag="p1")
                        v_ps = ps1.tile([128, AC_W], fp32, tag="p1")
                        cv_ps = ps1.tile([128, AC_W], fp32, tag="p1")
                        p1_window(ac, x_ts[ac], c_ts[ac], qk_ps, cv2_ps, v_ps, cv_ps)

                # ================= ATT =================
                with (
                    tc.tile_pool(name="att", bufs=2) as att_pool,
                    tc.tile_pool(name="nrm", bufs=2) as nrm_pool,
                    tc.tile_pool(name="s3o", bufs=3) as s3o_pool,
                ):
                    psA_cm = tc.tile_pool(name="psA", bufs=1, space="PSUM")
                    psA = psA_cm.__enter__()
                    sim_cm = tc.tile_pool(name="psSim", bufs=2, space="PSUM")
                    sim_pool = sim_cm.__enter__()
                    # PSUM same-bank accumulation groups conflict when open
                    # concurrently (the start bit claims the bank), so shared
                    # banks use DVE-memset + pure-accumulate (start=False).
                    # Banks 0-2: 16 ctx chunks (7+7+2) + pv_o slots in bank2;
                    # banks 3-6: sim double-buffer (2x 2-bank tiles);
                    # bank 7: PE-transpose dst for the post-A2A rebuild.
                    c_bank = [psA.tile([128, 512], fp32, name=f"c_bank{i}") for i in range(3)]
                    eT_ps2 = [psA.tile([128, 1024], bf16, name=f"eT_ps{i}")
                              for i in range(2)]

                    def c_chunk(g):
                        bk, off = divmod(g, 8)
                        return c_bank[bk][:, off * 64:(off + 1) * 64]

                    def pv_o_slot(it):
                        off = 130 + 65 * (it % 2)
                        return c_bank[2][:, off:off + 65]


                    post_tiles = {}

                    def posts_load(b, h):
                        # load issued right after the (b,h) A2A on the same
                        # Pool queue (transfer starts the moment the A2A
                        # lands), then the block transposes ride the DMA xbar
                        # in the window before the NEXT collective is issued
                        # (xbar transposes serialize against in-flight A2As,
                        # but this window is collective-free).
                        # load + xbar transpose both ride the SP queue from
                        # A2A-fire time: the queue head-blocks on the A2A,
                        # which is harmless (nothing else needs SP during a
                        # flight), and both run back-to-back at landing.
                        view = cc_out[b][h].ap().rearrange("(g p) c -> p g c", p=128)
                        if b == 1 and h == 1:
                            # tail: split by t-half (even/odd g blocks) so
                            # the at2=0 output projections start after only
                            # half the chain
                            tps = []
                            for par in range(2):
                                tmp_h = att_pool.tile([128, 8, 128], bf16,
                                                      tag=f"postS{par}", name=f"tmpS{par}")
                                nc.sync.dma_start(out=tmp_h, in_=view[:, par::2, :])
                                tp_h = att_pool.tile([128, 8, 128], bf16,
                                                     tag=f"postST{par}", name=f"tpS{par}")
                                nc.sync.dma_start(
                                    out=tp_h,
                                    in_=tmp_h.rearrange("p g c -> p (g c)"),
                                    transpose=True)
                                tps.append(tp_h)
                            post_tiles[(b, h)] = tps
                            return
                        tmp_t = att_pool.tile([128, 16, 128], bf16, tag="post")
                        nc.sync.dma_start(out=tmp_t, in_=view)
                        tp_t = att_pool.tile([128, 16, 128], bf16, tag="postT")
                        nc.sync.dma_start(
                            out=tp_t, in_=tmp_t.rearrange("p g c -> p (g c)"),
                            transpose=True)
                        post_tiles[(b, h)] = tp_t

                    def posts_half(b, h, par):
                        # copies for one t-half of the split tail chain
                        tp_h = post_tiles[(b, h)][par]
                        hs = slice(h * 64, (h + 1) * 64)
                        ts = slice(par * 128, (par + 1) * 128)
                        nc.vector.tensor_copy(full_o[b][hs, :, ts], tp_h[0:64, :, :])
                        nc.vector.tensor_copy(full_c[b][hs, :, ts], tp_h[64:128, :, :])

                    def posts_for(b, h, on_dve=False):
                        # post-A2A: copy the xbar-transposed [d, tok] blocks
                        # into the merged-head tiles. Mid-stream copies go to
                        # the idle gpsimd engine so the DVE FIFO (which paces
                        # the eT chain) never waits on the A2A; the tail pair
                        # uses the then-idle DVE for latency.
                        tp_t = post_tiles.pop((b, h))
                        eng = nc.vector if on_dve else nc.gpsimd
                        hs = slice(h * 64, (h + 1) * 64)
                        eng.tensor_copy(
                            full_o[b].rearrange("p k t -> p (k t)")[hs, :],
                            tp_t[0:64, :, :].rearrange("p g c -> p (g c)"))
                        eng.tensor_copy(
                            full_c[b].rearrange("p k t -> p (k t)")[hs, :],
                            tp_t[64:128, :, :].rearrange("p g c -> p (g c)"))

                    units = [(0, 0), (0, 1), (1, 0), (1, 1)]
                    UPU = 2 * NBB          # 32 jw-units per (b, h)
                    TOT = 4 * UPU

                    def uparams(ug):
                        ui = ug // UPU
                        b, h = units[ui]
                        u = ug % UPU
                        it, jw = divmod(u, 2)
                        return ui, b, h, it, jw

                    e_tiles = {}
                    eT_tiles = {}

                    # sgs whose pv runs while an A2A is in flight must use PE
                    # transposes: tile serializes every DmaTransposeAnt against
                    # every collective, so xbar eT production stalls ~28us
                    # whenever an AllToAll is in the air. Unit 1 is exempt:
                    # its flight window is filled by the deferred batch-1 P1
                    # (pure matmul work), after which its xbar eTs flow.
                    PEW = 32
                    # pv lag: eT's DMA path (DGE delay + xfer + DMA-sem
                    # ~2.5-3.2us) must land before pv consumes it, else the
                    # in-order PE queue head-blocks. 4 sg periods of slack.
                    PVLAG = 4

                    def use_dma_eT(sg):
                        u, off = divmod(sg, UPU)
                        return u == 0 or off >= PEW

                    def emit_sim(sg):
                        ui, b, h, it, jw = uparams(sg)
                        statT = qkT_sb[h * 64:(h + 1) * 64, :]
                        movT = cqkT_sb[h * 64:(h + 1) * 64, :]
                        tp = (64, 0) if h == 1 else None
                        si0 = b * N + it * 128
                        sm0 = b * N + jw * 1024
                        e_sb = att_pool.tile([128, 1024], bf16, tag="e", bufs=6)
                        e_tiles[sg] = e_sb
                        for hf in range(2):
                            sim = sim_pool.tile([128, 512], fp32, tag="sim", bufs=3)
                            nc.tensor.matmul(
                                sim,
                                statT[:, si0:si0 + 128],
                                movT[:, sm0 + hf * 512:sm0 + (hf + 1) * 512],
                                start=True, stop=True, tile_position=tp,
                            )
                            nc.scalar.activation(
                                e_sb[:, hf * 512:(hf + 1) * 512], sim,
                                EXP, scale=SCALE)
                        if use_dma_eT(sg):
                            # eT via the DMA xbar: issued ~2 jw-units before
                            # its pv1 consumer, hiding DGE+transfer latency.
                            eT_sb = att_pool.tile([128, 8, 128], bf16, tag="eT", bufs=6)
                            eT_tiles[sg] = eT_sb
                            nc.sync.dma_start(out=eT_sb, in_=e_sb, transpose=True)

                    # one continuous pipeline across all four units: sims
                    # lead by one jw-unit, pv1 lags two, so neither the PE
                    # stream nor the activation queue ever drains.
                    nc.vector.memset(c_bank[0], 0.0)
                    nc.vector.memset(c_bank[1], 0.0)
                    nc.vector.memset(c_bank[2][:, 130:276], 0.0)
                    emit_sim(0)
                    pv_next = [0]
                    for ug in range(TOT + PVLAG):
                        if ug == 2 * UPU + 2:
                            # big output-projection weights: ride the A2A(0,1)
                            # flight window (needed only at stage3 time)
                            nc.sync.dma_start(out=wcout_sb, in_=wcout.ap().rearrange("(k p) g -> p k g", p=128))
                            nc.sync.dma_start(out=wout_sb, in_=wout.ap().rearrange("(k p) g -> p k g", p=128))
                        if ug == 2 * UPU:
                            posts_for(0, 0)
                        if ug == 2 * UPU + 24:
                            posts_for(0, 1)
                        if ug == 3 * UPU + 28:
                            posts_for(1, 0)
                        if ug + 1 < TOT:
                            emit_sim(ug + 1)
                        # pv lag: 4 for unit 0 (DMA-eT latency), 2 for the
                        # PE-transpose units -- retiring pvs earlier fires
                        # each A2A ~2 jw-units sooner (tail shifts with it)
                        while pv_next[0] < TOT and ug >= pv_next[0] + (
                                PVLAG if pv_next[0] < UPU else 2):
                            pu = pv_next[0]
                            pv_next[0] += 1
                            pui, pb, ph, pit, pjw = uparams(pu)
                            pvo = pv_o_slot(pit)
                            eT_prev = eT_tiles.pop(pu)
                            pcvv = cv0_sb if ph == 0 else cv1_sb
                            for jc in range(8):
                                nc.tensor.matmul(
                                    pvo,
                                    eT_prev[:, jc, :],
                                    pcvv[:, pb * NBB + pjw * 8 + jc, :],
                                    start=False, stop=(pjw == 1 and jc == 7),
                                    skip_group_check=True,
                                )
                            if pjw == 1:
                                rec1 = nrm_pool.tile([128, 1], fp32, tag="r1")
                                nc.vector.reciprocal(rec1, pvo[:, 64:65])
                                nc.vector.tensor_scalar_mul(
                                    oa_nat[ph][:, pb * NBB + pit, :],
                                    pvo[:, 0:64], rec1)
                                nc.vector.memset(pvo, 0.0)
                                if pit == NBB - 1:
                                    # last pv1 of unit pui retired: bounce+A2A
                                    ccv = cc_in[pb][ph].ap().rearrange(
                                        "(bi p) c -> p bi c", p=128)
                                    bs = slice(pb * NBB, (pb + 1) * NBB)
                                    nc.sync.dma_start(out=ccv[:, :, 0:64],
                                                      in_=oa_nat[ph][:, bs, :])
                                    nc.sync.dma_start(out=ccv[:, :, 64:128],
                                                      in_=ca_nat[ph][:, bs, :])
                                    if use_cc:
                                        nc.gpsimd.collective_compute(
                                            "AllToAll", mybir.AluOpType.bypass,
                                            replica_groups=[list(range(N_CORES))],
                                            ins=[cc_in[pb][ph].ap().opt()],
                                            outs=[cc_out[pb][ph].ap().opt()],
                                        )
                                    else:
                                        nc.gpsimd.dma_start(
                                            out=cc_out[pb][ph].ap(),
                                            in_=cc_in[pb][ph].ap())
                                    posts_load(pb, ph)
                        if ug < TOT:
                            ui, b, h, it, jw = uparams(ug)
                            vv = v0_sb if h == 0 else v1_sb
                            e_sb = e_tiles.pop(ug)
                            if not use_dma_eT(ug):
                                eT_sb = att_pool.tile([128, 8, 128], bf16, tag="eT", bufs=6)
                                eT_tiles[ug] = eT_sb
                                eTp = eT_ps2[ug % 2]
                                for jc in range(8):
                                    nc.tensor.transpose(
                                        eTp[:, jc * 128:(jc + 1) * 128],
                                        e_sb[:, jc * 128:(jc + 1) * 128],
                                        ident)
                                nc.vector.tensor_copy(
                                    eT_sb,
                                    eTp.rearrange("p (c m) -> p c m", m=128))
                            for jc in range(8):
                                nc.tensor.matmul(
                                    c_chunk(jw * 8 + jc),
                                    e_sb[:, jc * 128:(jc + 1) * 128],
                                    vv[:, b * NBB + it, 0:64],
                                    start=False, stop=(it == NBB - 1),
                                    skip_group_check=True,
                                )
                                # per-chunk colsum (ctx normalizer): 1-col MM
                                g = jw * 8 + jc
                                nc.tensor.matmul(
                                    c_bank[2][:, 260 + g:261 + g],
                                    e_sb[:, jc * 128:(jc + 1) * 128],
                                    v0_sb[:, 0, 64:65],
                                    start=False, stop=(it == NBB - 1),
                                    skip_group_check=True,
                                )
                            cstg = None
                            if it == NBB - 1 and jw == 1:
                                # all ctx chunks of unit ui stopped: stage and
                                # re-zero the banks right away so the next
                                # unit's first pv2 isn't gated on the muls
                                cstg = nrm_pool.tile([128, 1040], fp32, tag="cstg")
                                nc.vector.tensor_copy(cstg[:, 0:512], c_bank[0])
                                nc.vector.memset(c_bank[0], 0.0)
                                nc.vector.tensor_copy(cstg[:, 512:1024], c_bank[1])
                                nc.vector.memset(c_bank[1], 0.0)
                                nc.vector.tensor_copy(cstg[:, 1024:1040], c_bank[2][:, 260:276])
                                nc.vector.memset(c_bank[2][:, 260:276], 0.0)
                            if cstg is not None:
                                cstg_v = cstg[:, 0:1024].rearrange("p (c w) -> p c w", w=64)
                                rec2 = nrm_pool.tile([128, 16], fp32, tag="r2")
                                nc.vector.reciprocal(rec2, cstg[:, 1024:1040])
                                for g in range(16):
                                    nc.vector.tensor_scalar_mul(
                                        ca_nat[h][:, b * NBB + g, :],
                                        cstg_v[:, g, :], rec2[:, g:g + 1])

                    # ============ tail ============
                    sim_cm.__exit__(None, None, None)
                    psA_cm.__exit__(None, None, None)
                    with tc.tile_pool(name="ps3t", bufs=1, space="PSUM") as ps3t:
                        def stage3_tail(b, full_t, w_sb, dst, at2):
                            ps3 = ps3t.tile([128, D], fp32, tag="s3b1", bufs=3)
                            for kc in range(KC):
                                for g in range(2):
                                    nc.tensor.matmul(
                                        ps3[:, g * 512:(g + 1) * 512],
                                        full_t[:, kc, at2 * 128:(at2 + 1) * 128],
                                        w_sb[:, kc, g * 512:(g + 1) * 512],
                                        start=(kc == 0), stop=(kc == KC - 1),
                                        skip_group_check=True,
                                    )
                            o_sb = s3o_pool.tile([128, D], bf16, tag="ot")
                            rows = slice(b * 256 + at2 * 128, b * 256 + (at2 + 1) * 128)
                            for g in range(2):
                                gs = slice(g * 512, (g + 1) * 512)
                                nc.vector.tensor_copy(o_sb[:, gs], ps3[:, gs])
                                nc.sync.dma_start(out=dst.ap()[rows, gs], in_=o_sb[:, gs])

                        # batch-0 projections run under the last collective
                        for at2 in range(2):
                            stage3_tail(0, full_c[0], wcout_sb, ctx_sl, at2)
                        for at2 in range(2):
                            stage3_tail(0, full_o[0], wout_sb, out_sl, at2)

                        # batch-1: even t-half (at2=0) consumes the first
                        # half of the split posts chain, odd the second
                        posts_half(1, 1, 0)
                        stage3_tail(1, full_c[1], wcout_sb, ctx_sl, 0)
                        stage3_tail(1, full_o[1], wout_sb, out_sl, 0)
                        posts_half(1, 1, 1)
                        stage3_tail(1, full_c[1], wcout_sb, ctx_sl, 1)
                        stage3_tail(1, full_o[1], wout_sb, out_sl, 1)

                if debug:
                        for nm, t in (("qkT", qkT_sb), ("cqkT", cqkT_sb),
                                      ("v0", v0_sb), ("cv0", cv0_sb),
                                      ("v1", v1_sb), ("cv1", cv1_sb),
                                      ("oa0", oa_nat[0]), ("oa1", oa_nat[1]),
                                      ("ca0", ca_nat[0]), ("ca1", ca_nat[1]),
                                      ("fo0", full_o[0]), ("fc0", full_c[0]),
                                      ("fo1", full_o[1]), ("fc1", full_c[1])):
                            sh = [t.shape[0], int(np.prod(t.shape[1:]))]
                            d = nc.dram_tensor(f"dbg_{nm}", sh, bf16, kind="ExternalOutput")
                            nc.sync.dma_start(out=d.ap(), in_=t.rearrange(
                                "p a b -> p (a b)") if len(t.shape) == 3 else t)
    nc.compile()
    return nc


_NC_CACHE = {}


def _get_nc():
    if "nc" not in _NC_CACHE:
        _NC_CACHE["nc"] = build_nc()
    return _NC_CACHE["nc"]


def _run(in_maps, trace=False):
    from concourse.bass_utils import run_bass_kernel_spmd
    nc = _get_nc()
    return run_bass_kernel_spmd(nc, in_maps, core_ids=list(range(N_CORES)), trace=trace)


def prepare_in_maps(x, context, w_qk, w_v, w_cqk, w_cv, w_out, w_cout):
    x = np.asarray(x, dtype=np.float32)
    context = np.asarray(context, dtype=np.float32)
    xT = np.ascontiguousarray(x.reshape(A, D).T).astype(BF16)
    cT = np.ascontiguousarray(context.reshape(A, D).T).astype(BF16)
    wout_b = np.asarray(w_out, np.float32).astype(BF16)
    wcout_b = np.asarray(w_cout, np.float32).astype(BF16)
    in_maps = []
    for c in range(N_CORES):
        cs = slice(c * HD, (c + 1) * HD)
        in_maps.append({
            "xT": xT,
            "cT": cT,
            "wqk": np.ascontiguousarray(np.asarray(w_qk, np.float32)[:, cs]).astype(BF16),
            "wv": np.ascontiguousarray(np.asarray(w_v, np.float32)[:, cs]).astype(BF16),
            "wcqk": np.ascontiguousarray(np.asarray(w_cqk, np.float32)[:, cs]).astype(BF16),
            "wcv": np.ascontiguousarray(np.asarray(w_cv, np.float32)[:, cs]).astype(BF16),
            "wout": wout_b,
            "wcout": wcout_b,
        })
    return in_maps


def assemble(results):
    out = np.empty((B, N, D), np.float32)
    ctx = np.empty((B, N, D), np.float32)
    for c in range(N_CORES):
        o = results[c]["out_sl"].reshape(B, SL, D)
        k = results[c]["ctx_sl"].reshape(B, SL, D)
        out[:, c * SL:(c + 1) * SL, :] = o
        ctx[:, c * SL:(c + 1) * SL, :] = k
    return out, ctx


def kernel(x, context, w_qk, w_v, w_cqk, w_cv, w_out, w_cout):
    in_maps = prepare_in_maps(x, context, w_qk, w_v, w_cqk, w_cv, w_out, w_cout)
    res = _run(in_maps)
    return assemble(res.results)

